# revision 6
# baseline (speedup 1.0000x reference)
"""Trainium2 Bass kernel for multi-head attention.

Problem: B=4, H=16, S=2048, D=128, fp32.
  scores = (q @ k^T) / sqrt(128); probs = softmax(scores, -1); out = probs @ v

Sharding: 64 (b,h) pairs -> 8 cores x 8 pairs. Fully independent per pair.

v3 layout (all-bf16 datapath; T-layout per (b,h) pair; s in halves of 1024):
  qT, kT: [D=128, S=2048] bf16 in SBUF. Scores live in a single PSUM ring
  tile [128, 3072] (6 banks = 3 rotating slots of 1024 fp32); the PV
  accumulator [128,1024] takes the last 2 banks.
  t-tiles are processed in PAIRS (2 x 128 keys):
    scoresT[t, s] = kT.T @ qT        (PE, 2 matmuls N=512 per tile)
    expT pair -> bf16                (ACT: ONE N=2048 instr when the two
                                      slots are ring-adjacent, else 2x
                                      N=1024 -- amortizes the ~160ns/instr
                                      ACT overhead)
    outT[d, s] += v_tile.T @ expT    (PE, 2 matmuls N=512 per tile, accum)
    eacc2 += expT pair               (DVE bf16 [128,2048] adds, 2x mode)
  Tail per half: fold eacc2 halves (DVE), collapse 128 t-partitions with
  partition_all_reduce on the idle GPSIMD/Pool engine (replaces the PE
  ones-matmul of v1/v2 and frees its PSUM slot), reciprocal (DVE approx),
  out = outT * rec (DVE), DMA out fp32.
  A global tile->slot counter rotates the ring across halves so each
  half's first QKs hit slots freed 2+ pairs earlier (no boundary stall).

bf16 end-to-end error vs the fp32 reference is ~3e-3 relative (threshold
2e-2).
"""

import sys

sys.path.insert(0, "/opt/trn_rl_repo")

import numpy as np

B, H, S, D = 4, 16, 2048, 128
N_CORES = 8
BH = B * H                      # 64 pairs
BH_PER_CORE = BH // N_CORES     # 8
T_TILES = S // 128              # 16
T_PAIRS = T_TILES // 2          # 8
S_HALF = S // 2                 # 1024
SCALE = float(D) ** -0.5

_cache = {}


def _build_program():
    import concourse.tile as tile
    from concourse import bacc, mybir
    from concourse import bass_isa

    F32 = mybir.dt.float32
    BF16 = mybir.dt.bfloat16

    nc = bacc.Bacc("TRN2", target_bir_lowering=False, debug=False)

    qt = nc.dram_tensor("qt", [BH_PER_CORE, D, S], BF16, kind="ExternalInput")
    kt = nc.dram_tensor("kt", [BH_PER_CORE, D, S], BF16, kind="ExternalInput")
    # v pre-shuffled on host to [p, t, d] so the load is fully contiguous
    v = nc.dram_tensor("v", [BH_PER_CORE, 128, T_TILES * D], BF16, kind="ExternalInput")
    ot = nc.dram_tensor("ot", [BH_PER_CORE, D, S], F32, kind="ExternalOutput")

    with tile.TileContext(nc) as tc:
        with (
            tc.tile_pool(name="rin", bufs=2) as rin,
            tc.tile_pool(name="exps", bufs=6) as exps,
            tc.tile_pool(name="accp", bufs=2) as accp,
            tc.tile_pool(name="folds", bufs=2) as folds,
            tc.tile_pool(name="outs", bufs=4) as outs,
            tc.tile_pool(name="psc", bufs=1, space="PSUM") as psc,
            tc.tile_pool(name="pacc", bufs=1, space="PSUM") as pacc,
        ):
            # Persistent 6-bank score ring: 3 slots x 1024 fp32 columns.
            ring = psc.tile([128, 3 * S_HALF], F32, name="ring")
            slot_ctr = [0]  # global tile->slot rotation across halves

            for i in range(BH_PER_CORE):
                q_r = rin.tile([D, S], BF16, tag="q_r")
                k_r = rin.tile([D, S], BF16, tag="k_r")
                v_r = rin.tile([128, T_TILES, D], BF16, tag="v_r")
                # order: what the first tiles need comes first
                nc.sync.dma_start(out=k_r[:, :128], in_=kt[i, :, :128])
                nc.sync.dma_start(out=q_r[:, :S_HALF], in_=qt[i, :, :S_HALF])
                nc.sync.dma_start(out=k_r[:, 128:S_HALF], in_=kt[i, :, 128:S_HALF])
                nc.sync.dma_start(
                    out=v_r[:], in_=v[i].rearrange("p (t d) -> p t d", t=T_TILES)
                )
                nc.sync.dma_start(out=q_r[:, S_HALF:], in_=qt[i, :, S_HALF:])
                nc.sync.dma_start(out=k_r[:, S_HALF:], in_=kt[i, :, S_HALF:])

                for h in range(2):
                    s0 = h * S_HALF
                    oacc = pacc.tile([128, S_HALF], F32, tag="oacc")
                    eacc2 = accp.tile([128, 2 * S_HALF], BF16, tag="eacc2")

                    epairs = [None] * T_PAIRS

                    def qk(t, slot):
                        for c in range(0, S_HALF, 512):
                            nc.tensor.matmul(
                                ring[:, slot * S_HALF + c : slot * S_HALF + c + 512],
                                k_r[:, t * 128 : (t + 1) * 128],
                                q_r[:, s0 + c : s0 + c + 512],
                                start=True,
                                stop=True,
                            )

                    def consume(j):
                        # PV for the two tiles of pair j
                        ep = epairs[j]
                        for half_idx, t in ((0, 2 * j), (1, 2 * j + 1)):
                            for c in range(0, S_HALF, 512):
                                nc.tensor.matmul(
                                    oacc[:, c : c + 512],
                                    v_r[:, t, :],
                                    ep[:, half_idx * S_HALF + c : half_idx * S_HALF + c + 512],
                                    start=(t == 0),
                                    stop=(t == T_TILES - 1),
                                )
                        if j == 1:
                            nc.vector.tensor_add(
                                eacc2[:], epairs[0][:], epairs[1][:]
                            )
                        elif j > 1:
                            nc.vector.tensor_add(eacc2[:], eacc2[:], ep[:])

                    for j in range(T_PAIRS):
                        sl0 = slot_ctr[0] % 3
                        sl1 = (slot_ctr[0] + 1) % 3
                        slot_ctr[0] += 2
                        qk(2 * j, sl0)
                        qk(2 * j + 1, sl1)
                        ep = exps.tile(
                            [128, 2 * S_HALF], BF16, tag="ep", name=f"ep_{j}"
                        )
                        if sl1 == sl0 + 1:
                            # ring-adjacent: one big activation
                            nc.scalar.activation(
                                ep[:],
                                ring[:, sl0 * S_HALF : (sl0 + 2) * S_HALF],
                                mybir.ActivationFunctionType.Exp,
                                scale=SCALE,
                            )
                        else:
                            # wrapped pair: two activations
                            nc.scalar.activation(
                                ep[:, 0:S_HALF],
                                ring[:, sl0 * S_HALF : (sl0 + 1) * S_HALF],
                                mybir.ActivationFunctionType.Exp,
                                scale=SCALE,
                            )
                            nc.scalar.activation(
                                ep[:, S_HALF : 2 * S_HALF],
                                ring[:, sl1 * S_HALF : (sl1 + 1) * S_HALF],
                                mybir.ActivationFunctionType.Exp,
                                scale=SCALE,
                            )
                        epairs[j] = ep
                        if j >= 1:
                            consume(j - 1)
                    consume(T_PAIRS - 1)

                    fold = folds.tile([128, S_HALF], BF16, tag="fold")
                    nc.vector.tensor_add(
                        fold[:], eacc2[:, 0:S_HALF], eacc2[:, S_HALF:]
                    )
                    sums = outs.tile([128, S_HALF], F32, tag="sums")
                    nc.gpsimd.partition_all_reduce(
                        sums[:], fold[:], channels=128,
                        reduce_op=bass_isa.ReduceOp.add,
                    )
                    rec = outs.tile([128, S_HALF], F32, tag="rec")
                    nc.vector.reciprocal_approx_fast(out=rec[:], in_=sums[:])
                    osb = outs.tile([128, S_HALF], F32, tag="osb")
                    nc.vector.tensor_mul(osb[:], oacc[:], rec[:])
                    nc.sync.dma_start(out=ot[i, :, s0 : s0 + S_HALF], in_=osb[:])

    nc.finalize()
    return nc


def _get_program():
    if "nc" not in _cache:
        _cache["nc"] = _build_program()
    return _cache["nc"]


def kernel(q: np.ndarray, k: np.ndarray, v: np.ndarray) -> np.ndarray:
    import ml_dtypes
    from concourse.bass_utils import run_bass_kernel_spmd

    nc = _get_program()

    bf16 = ml_dtypes.bfloat16
    q4 = np.asarray(q, dtype=np.float32).reshape(BH, S, D)
    k4 = np.asarray(k, dtype=np.float32).reshape(BH, S, D)
    v4 = np.asarray(v, dtype=np.float32).reshape(BH, S, D)

    in_maps = []
    for core in range(N_CORES):
        sl = slice(core * BH_PER_CORE, (core + 1) * BH_PER_CORE)
        in_maps.append(
            {
                "qt": np.ascontiguousarray(
                    q4[sl].transpose(0, 2, 1).astype(bf16)
                ),
                "kt": np.ascontiguousarray(
                    k4[sl].transpose(0, 2, 1).astype(bf16)
                ),
                # [i, t*128+p, d] -> [i, p, t*128+d]
                "v": np.ascontiguousarray(
                    v4[sl]
                    .reshape(BH_PER_CORE, T_TILES, 128, D)
                    .transpose(0, 2, 1, 3)
                    .reshape(BH_PER_CORE, 128, T_TILES * D)
                    .astype(bf16)
                ),
            }
        )

    res = run_bass_kernel_spmd(nc, in_maps, core_ids=list(range(N_CORES)))

    out = np.empty((BH, S, D), dtype=np.float32)
    for core in range(N_CORES):
        ot = res.results[core]["ot"]  # [BH_PER_CORE, D, S]
        out[core * BH_PER_CORE : (core + 1) * BH_PER_CORE] = ot.transpose(0, 2, 1)
    return out.reshape(B, H, S, D)


# revision 7
# speedup vs baseline: 1.0019x; 1.0019x over previous
"""Trainium2 Bass kernel for multi-head attention.

Problem: B=4, H=16, S=2048, D=128, fp32.
  scores = (q @ k^T) / sqrt(128); probs = softmax(scores, -1); out = probs @ v

Sharding: 64 (b,h) pairs -> 8 cores x 8 pairs. Fully independent per pair.

v3 layout (all-bf16 datapath; T-layout per (b,h) pair; s in halves of 1024):
  qT, kT: [D=128, S=2048] bf16 in SBUF. Scores live in a single PSUM ring
  tile [128, 3072] (6 banks = 3 rotating slots of 1024 fp32); the PV
  accumulator [128,1024] takes the last 2 banks.
  t-tiles are processed in PAIRS (2 x 128 keys):
    scoresT[t, s] = kT.T @ qT        (PE, 2 matmuls N=512 per tile)
    expT pair -> bf16                (ACT: ONE N=2048 instr when the two
                                      slots are ring-adjacent, else 2x
                                      N=1024 -- amortizes the ~160ns/instr
                                      ACT overhead)
    outT[d, s] += v_tile.T @ expT    (PE, 2 matmuls N=512 per tile, accum)
    eacc2 += expT pair               (DVE bf16 [128,2048] adds, 2x mode)
  Tail per half: fold eacc2 halves (DVE), collapse 128 t-partitions with
  partition_all_reduce on the idle GPSIMD/Pool engine (replaces the PE
  ones-matmul of v1/v2 and frees its PSUM slot), reciprocal (DVE approx),
  out = outT * rec (DVE), DMA out fp32.
  A global tile->slot counter rotates the ring across halves so each
  half's first QKs hit slots freed 2+ pairs earlier (no boundary stall).

bf16 end-to-end error vs the fp32 reference is ~3e-3 relative (threshold
2e-2).
"""

import sys

sys.path.insert(0, "/opt/trn_rl_repo")

import numpy as np

B, H, S, D = 4, 16, 2048, 128
N_CORES = 8
BH = B * H                      # 64 pairs
BH_PER_CORE = BH // N_CORES     # 8
T_TILES = S // 128              # 16
T_PAIRS = T_TILES // 2          # 8
S_HALF = S // 2                 # 1024
SCALE = float(D) ** -0.5

_cache = {}


def _build_program():
    import concourse.tile as tile
    from concourse import bacc, mybir
    from concourse import bass_isa

    F32 = mybir.dt.float32
    BF16 = mybir.dt.bfloat16

    nc = bacc.Bacc("TRN2", target_bir_lowering=False, debug=False)

    qt = nc.dram_tensor("qt", [BH_PER_CORE, D, S], BF16, kind="ExternalInput")
    kt = nc.dram_tensor("kt", [BH_PER_CORE, D, S], BF16, kind="ExternalInput")
    # v pre-shuffled on host to [p, t, d] so the load is fully contiguous
    v = nc.dram_tensor("v", [BH_PER_CORE, 128, T_TILES * D], BF16, kind="ExternalInput")
    ot = nc.dram_tensor("ot", [BH_PER_CORE, D, S], F32, kind="ExternalOutput")

    with tile.TileContext(nc) as tc:
        with (
            tc.tile_pool(name="rin", bufs=2) as rin,
            tc.tile_pool(name="exps", bufs=12) as exps,
            tc.tile_pool(name="accp", bufs=2) as accp,
            tc.tile_pool(name="folds", bufs=3) as folds,
            tc.tile_pool(name="outs", bufs=6) as outs,
            tc.tile_pool(name="psc", bufs=1, space="PSUM") as psc,
            tc.tile_pool(name="pacc", bufs=1, space="PSUM") as pacc,
        ):
            # Persistent 6-bank score ring: 3 slots x 1024 fp32 columns.
            ring = psc.tile([128, 3 * S_HALF], F32, name="ring")
            slot_ctr = [0]  # global tile->slot rotation across halves

            for i in range(BH_PER_CORE):
                q_r = rin.tile([D, S], BF16, tag="q_r")
                k_r = rin.tile([D, S], BF16, tag="k_r")
                v_r = rin.tile([128, T_TILES, D], BF16, tag="v_r")
                # order: what the first tiles need comes first
                nc.sync.dma_start(out=k_r[:, :128], in_=kt[i, :, :128])
                nc.sync.dma_start(out=q_r[:, :S_HALF], in_=qt[i, :, :S_HALF])
                nc.sync.dma_start(out=k_r[:, 128:S_HALF], in_=kt[i, :, 128:S_HALF])
                nc.sync.dma_start(
                    out=v_r[:], in_=v[i].rearrange("p (t d) -> p t d", t=T_TILES)
                )
                nc.sync.dma_start(out=q_r[:, S_HALF:], in_=qt[i, :, S_HALF:])
                nc.sync.dma_start(out=k_r[:, S_HALF:], in_=kt[i, :, S_HALF:])

                for h in range(2):
                    s0 = h * S_HALF
                    oacc = pacc.tile([128, S_HALF], F32, tag="oacc")
                    eacc2 = accp.tile([128, 2 * S_HALF], BF16, tag="eacc2")

                    epairs = [None] * T_PAIRS

                    def qk(t, slot):
                        for c in range(0, S_HALF, 512):
                            nc.tensor.matmul(
                                ring[:, slot * S_HALF + c : slot * S_HALF + c + 512],
                                k_r[:, t * 128 : (t + 1) * 128],
                                q_r[:, s0 + c : s0 + c + 512],
                                start=True,
                                stop=True,
                            )

                    def consume(j):
                        # PV for the two tiles of pair j
                        ep = epairs[j]
                        for half_idx, t in ((0, 2 * j), (1, 2 * j + 1)):
                            for c in range(0, S_HALF, 512):
                                nc.tensor.matmul(
                                    oacc[:, c : c + 512],
                                    v_r[:, t, :],
                                    ep[:, half_idx * S_HALF + c : half_idx * S_HALF + c + 512],
                                    start=(t == 0),
                                    stop=(t == T_TILES - 1),
                                )
                        if j == 1:
                            nc.vector.tensor_add(
                                eacc2[:], epairs[0][:], epairs[1][:]
                            )
                        elif j > 1:
                            nc.vector.tensor_add(eacc2[:], eacc2[:], ep[:])

                    for j in range(T_PAIRS):
                        sl0 = slot_ctr[0] % 3
                        sl1 = (slot_ctr[0] + 1) % 3
                        slot_ctr[0] += 2
                        qk(2 * j, sl0)
                        qk(2 * j + 1, sl1)
                        ep = exps.tile(
                            [128, 2 * S_HALF], BF16, tag="ep", name=f"ep_{j}"
                        )
                        if sl1 == sl0 + 1:
                            # ring-adjacent: one big activation
                            nc.scalar.activation(
                                ep[:],
                                ring[:, sl0 * S_HALF : (sl0 + 2) * S_HALF],
                                mybir.ActivationFunctionType.Exp,
                                scale=SCALE,
                            )
                        else:
                            # wrapped pair: two activations
                            nc.scalar.activation(
                                ep[:, 0:S_HALF],
                                ring[:, sl0 * S_HALF : (sl0 + 1) * S_HALF],
                                mybir.ActivationFunctionType.Exp,
                                scale=SCALE,
                            )
                            nc.scalar.activation(
                                ep[:, S_HALF : 2 * S_HALF],
                                ring[:, sl1 * S_HALF : (sl1 + 1) * S_HALF],
                                mybir.ActivationFunctionType.Exp,
                                scale=SCALE,
                            )
                        epairs[j] = ep
                        if j >= 1:
                            consume(j - 1)
                    consume(T_PAIRS - 1)

                    fold = folds.tile([128, S_HALF], BF16, tag="fold")
                    nc.vector.tensor_add(
                        fold[:], eacc2[:, 0:S_HALF], eacc2[:, S_HALF:]
                    )
                    sums = outs.tile([128, S_HALF], F32, tag="sums")
                    nc.gpsimd.partition_all_reduce(
                        sums[:], fold[:], channels=128,
                        reduce_op=bass_isa.ReduceOp.add,
                    )
                    rec = outs.tile([128, S_HALF], F32, tag="rec")
                    nc.vector.reciprocal_approx_fast(out=rec[:], in_=sums[:])
                    osb = outs.tile([128, S_HALF], F32, tag="osb")
                    nc.vector.tensor_mul(osb[:], oacc[:], rec[:])
                    nc.sync.dma_start(out=ot[i, :, s0 : s0 + S_HALF], in_=osb[:])

    nc.finalize()
    return nc


def _get_program():
    if "nc" not in _cache:
        _cache["nc"] = _build_program()
    return _cache["nc"]


def kernel(q: np.ndarray, k: np.ndarray, v: np.ndarray) -> np.ndarray:
    import ml_dtypes
    from concourse.bass_utils import run_bass_kernel_spmd

    nc = _get_program()

    bf16 = ml_dtypes.bfloat16
    q4 = np.asarray(q, dtype=np.float32).reshape(BH, S, D)
    k4 = np.asarray(k, dtype=np.float32).reshape(BH, S, D)
    v4 = np.asarray(v, dtype=np.float32).reshape(BH, S, D)

    in_maps = []
    for core in range(N_CORES):
        sl = slice(core * BH_PER_CORE, (core + 1) * BH_PER_CORE)
        in_maps.append(
            {
                "qt": np.ascontiguousarray(
                    q4[sl].transpose(0, 2, 1).astype(bf16)
                ),
                "kt": np.ascontiguousarray(
                    k4[sl].transpose(0, 2, 1).astype(bf16)
                ),
                # [i, t*128+p, d] -> [i, p, t*128+d]
                "v": np.ascontiguousarray(
                    v4[sl]
                    .reshape(BH_PER_CORE, T_TILES, 128, D)
                    .transpose(0, 2, 1, 3)
                    .reshape(BH_PER_CORE, 128, T_TILES * D)
                    .astype(bf16)
                ),
            }
        )

    res = run_bass_kernel_spmd(nc, in_maps, core_ids=list(range(N_CORES)))

    out = np.empty((BH, S, D), dtype=np.float32)
    for core in range(N_CORES):
        ot = res.results[core]["ot"]  # [BH_PER_CORE, D, S]
        out[core * BH_PER_CORE : (core + 1) * BH_PER_CORE] = ot.transpose(0, 2, 1)
    return out.reshape(B, H, S, D)


# revision 8
# speedup vs baseline: 1.3280x; 1.3255x over previous
"""Trainium2 Bass kernel for multi-head attention.

Problem: B=4, H=16, S=2048, D=128, fp32.
  scores = (q @ k^T) / sqrt(128); probs = softmax(scores, -1); out = probs @ v

Sharding: 64 (b,h) pairs -> 8 cores x 8 pairs. Fully independent per pair.

v4 layout (all-bf16 datapath; T-layout per (b,h) pair; s in halves of 1024):
  qT, kT: [D=128, S=2048] bf16 in SBUF. For each t-tile (128 keys):
    scoresT[t, s] = kT[:, t-tile].T @ qT   (PE, 2 matmuls N=512, bf16)
    expT = exp(scoresT / sqrt(D)) -> bf16  (ACT, fused scale, psum->sbuf)
    outT[d, s] += v_tile.T @ expT          (PE, 2 matmuls N=512, accum)
    eacc += expT                           (DVE bf16 adds, 2x mode)
  Tail per half: collapse the 128 t-partitions of eacc with
  partition_all_reduce on the otherwise-idle GPSIMD/Pool engine (replaces
  the v1 PE ones-matmul and its PSUM-slot conflict -- the scores pool
  rotation then lines up so the next half's QKs only wait on exps that
  already finished), reciprocal (DVE approx), out = outT * rec (DVE),
  DMA out fp32.

PSUM: 3 score slots x 2 banks + outT accumulator 2 banks = 8 banks.
The t-loop is software-pipelined by one tile; the exps pool is 12 deep so
the per-half denominator chain (allreduce ~5us) never back-pressures ACT.

bf16 end-to-end error vs the fp32 reference is ~3e-3 relative (threshold
2e-2).
"""

import sys

sys.path.insert(0, "/opt/trn_rl_repo")

import numpy as np

B, H, S, D = 4, 16, 2048, 128
N_CORES = 8
BH = B * H                      # 64 pairs
BH_PER_CORE = BH // N_CORES     # 8
T_TILES = S // 128              # 16
S_HALF = S // 2                 # 1024
SCALE = float(D) ** -0.5

_cache = {}


def _build_program():
    import concourse.tile as tile
    from concourse import bacc, mybir
    from concourse import bass_isa

    F32 = mybir.dt.float32
    BF16 = mybir.dt.bfloat16

    nc = bacc.Bacc("TRN2", target_bir_lowering=False, debug=False)

    qt = nc.dram_tensor("qt", [BH_PER_CORE, D, S], BF16, kind="ExternalInput")
    kt = nc.dram_tensor("kt", [BH_PER_CORE, D, S], BF16, kind="ExternalInput")
    # v pre-shuffled on host to [p, t, d] so the load is fully contiguous
    v = nc.dram_tensor("v", [BH_PER_CORE, 128, T_TILES * D], BF16, kind="ExternalInput")
    ot = nc.dram_tensor("ot", [BH_PER_CORE, D, S], F32, kind="ExternalOutput")

    with tile.TileContext(nc) as tc:
        with (
            tc.tile_pool(name="rin", bufs=2) as rin,
            tc.tile_pool(name="exps", bufs=12) as exps,
            tc.tile_pool(name="accp", bufs=2) as accp,
            tc.tile_pool(name="outs", bufs=6) as outs,
            tc.tile_pool(name="psc", bufs=3, space="PSUM") as psc,
            tc.tile_pool(name="pacc", bufs=1, space="PSUM") as pacc,
        ):
            for i in range(BH_PER_CORE):
                q_r = rin.tile([D, S], BF16, tag="q_r")
                k_r = rin.tile([D, S], BF16, tag="k_r")
                v_r = rin.tile([128, T_TILES, D], BF16, tag="v_r")
                # order: what the first tiles need comes first
                nc.sync.dma_start(out=k_r[:, :128], in_=kt[i, :, :128])
                nc.sync.dma_start(out=q_r[:, :512], in_=qt[i, :, :512])
                nc.sync.dma_start(out=q_r[:, 512:S_HALF], in_=qt[i, :, 512:S_HALF])
                nc.sync.dma_start(out=k_r[:, 128:S_HALF], in_=kt[i, :, 128:S_HALF])
                nc.sync.dma_start(
                    out=v_r[:], in_=v[i].rearrange("p (t d) -> p t d", t=T_TILES)
                )
                nc.sync.dma_start(out=q_r[:, S_HALF:], in_=qt[i, :, S_HALF:])
                nc.sync.dma_start(out=k_r[:, S_HALF:], in_=kt[i, :, S_HALF:])

                for h in range(2):
                    s0 = h * S_HALF
                    oacc = pacc.tile([128, S_HALF], F32, tag="oacc")
                    eacc = accp.tile([128, S_HALF], BF16, tag="eacc")

                    ets = [None] * T_TILES

                    def consume(t):
                        for c in range(0, S_HALF, 512):
                            nc.tensor.matmul(
                                oacc[:, c : c + 512],
                                v_r[:, t, :],
                                ets[t][:, c : c + 512],
                                start=(t == 0),
                                stop=(t == T_TILES - 1),
                            )
                        if t == 1:
                            nc.vector.tensor_add(eacc[:], ets[0][:], ets[1][:])
                        elif t > 1:
                            nc.vector.tensor_add(eacc[:], eacc[:], ets[t][:])

                    for t in range(T_TILES):
                        sc = psc.tile([128, S_HALF], F32, tag="sc")
                        for c in range(0, S_HALF, 512):
                            nc.tensor.matmul(
                                sc[:, c : c + 512],
                                k_r[:, t * 128 : (t + 1) * 128],
                                q_r[:, s0 + c : s0 + c + 512],
                                start=True,
                                stop=True,
                            )
                        ets[t] = exps.tile(
                            [128, S_HALF], BF16, tag="et", name=f"et_{t}"
                        )
                        nc.scalar.activation(
                            ets[t][:],
                            sc[:],
                            mybir.ActivationFunctionType.Exp,
                            scale=SCALE,
                        )
                        if t >= 1:
                            consume(t - 1)
                    consume(T_TILES - 1)

                    sums = outs.tile([128, S_HALF], F32, tag="sums")
                    nc.gpsimd.partition_all_reduce(
                        sums[:], eacc[:], channels=128,
                        reduce_op=bass_isa.ReduceOp.add,
                    )
                    rec = outs.tile([128, S_HALF], F32, tag="rec")
                    nc.vector.reciprocal_approx_fast(out=rec[:], in_=sums[:])
                    osb = outs.tile([128, S_HALF], F32, tag="osb")
                    nc.vector.tensor_mul(osb[:], oacc[:], rec[:])
                    nc.sync.dma_start(out=ot[i, :, s0 : s0 + S_HALF], in_=osb[:])

    nc.finalize()
    return nc


def _get_program():
    if "nc" not in _cache:
        _cache["nc"] = _build_program()
    return _cache["nc"]


def kernel(q: np.ndarray, k: np.ndarray, v: np.ndarray) -> np.ndarray:
    import ml_dtypes
    from concourse.bass_utils import run_bass_kernel_spmd

    nc = _get_program()

    bf16 = ml_dtypes.bfloat16
    q4 = np.asarray(q, dtype=np.float32).reshape(BH, S, D)
    k4 = np.asarray(k, dtype=np.float32).reshape(BH, S, D)
    v4 = np.asarray(v, dtype=np.float32).reshape(BH, S, D)

    in_maps = []
    for core in range(N_CORES):
        sl = slice(core * BH_PER_CORE, (core + 1) * BH_PER_CORE)
        in_maps.append(
            {
                "qt": np.ascontiguousarray(
                    q4[sl].transpose(0, 2, 1).astype(bf16)
                ),
                "kt": np.ascontiguousarray(
                    k4[sl].transpose(0, 2, 1).astype(bf16)
                ),
                # [i, t*128+p, d] -> [i, p, t*128+d]
                "v": np.ascontiguousarray(
                    v4[sl]
                    .reshape(BH_PER_CORE, T_TILES, 128, D)
                    .transpose(0, 2, 1, 3)
                    .reshape(BH_PER_CORE, 128, T_TILES * D)
                    .astype(bf16)
                ),
            }
        )

    res = run_bass_kernel_spmd(nc, in_maps, core_ids=list(range(N_CORES)))

    out = np.empty((BH, S, D), dtype=np.float32)
    for core in range(N_CORES):
        ot = res.results[core]["ot"]  # [BH_PER_CORE, D, S]
        out[core * BH_PER_CORE : (core + 1) * BH_PER_CORE] = ot.transpose(0, 2, 1)
    return out.reshape(B, H, S, D)


# revision 9
# speedup vs baseline: 1.8308x; 1.3786x over previous
"""Trainium2 Bass kernel for multi-head attention.

Problem: B=4, H=16, S=2048, D=128, fp32.
  scores = (q @ k^T) / sqrt(128); probs = softmax(scores, -1); out = probs @ v

Sharding: 64 (b,h) pairs -> 8 cores x 8 pairs. Fully independent per pair.

v5 layout (all-bf16 datapath; T-layout per (b,h) pair; s in halves of 1024):
  qT, kT: [D=128, S=2048] bf16 in SBUF. For each t-tile (128 keys):
    scoresT[t, s] = kT[:, t-tile].T @ qT   (PE, 2 matmuls N=512, bf16)
    expT = exp(scoresT / sqrt(D)) -> bf16  (ACT, fused scale, psum->sbuf)
    outT[d, s] += v_tile.T @ expT          (PE, 2 matmuls N=512, accum)
    eacc += expT                           (DVE bf16 adds, 2x mode)
  Tail per half, pipelined in 512-column chunks so the psc slot that
  doubles as the sums buffer is released ~1us after the last exp (the
  next half's 3rd QK reuses it; an unchunked chain stalled ACT ~1.9us
  per half in v1/v2):
    chunk c: eacc_c += expT15_c (DVE) -> sums_c = ones.T @ eacc_c (PE)
             -> rec_c = ~1/sums_c (DVE approx) -> out_c = outT_c * rec_c.

PSUM: 3 score slots x 2 banks (one doubles as sums at the tail) + outT
accumulator 2 banks = 8 banks. t-loop software-pipelined by one tile;
exps pool 12 deep so the tail chain never back-pressures ACT.

bf16 end-to-end error vs the fp32 reference is ~3e-3 relative (threshold
2e-2).
"""

import sys

sys.path.insert(0, "/opt/trn_rl_repo")

import numpy as np

B, H, S, D = 4, 16, 2048, 128
N_CORES = 8
BH = B * H                      # 64 pairs
BH_PER_CORE = BH // N_CORES     # 8
T_TILES = S // 128              # 16
S_HALF = S // 2                 # 1024
SCALE = float(D) ** -0.5

_cache = {}


def _build_program():
    import concourse.tile as tile
    from concourse import bacc, mybir

    F32 = mybir.dt.float32
    BF16 = mybir.dt.bfloat16

    nc = bacc.Bacc("TRN2", target_bir_lowering=False, debug=False)

    qt = nc.dram_tensor("qt", [BH_PER_CORE, D, S], BF16, kind="ExternalInput")
    kt = nc.dram_tensor("kt", [BH_PER_CORE, D, S], BF16, kind="ExternalInput")
    # v pre-shuffled on host to [p, t, d] so the load is fully contiguous
    v = nc.dram_tensor("v", [BH_PER_CORE, 128, T_TILES * D], BF16, kind="ExternalInput")
    ot = nc.dram_tensor("ot", [BH_PER_CORE, D, S], F32, kind="ExternalOutput")

    with tile.TileContext(nc) as tc:
        with (
            tc.tile_pool(name="const", bufs=1) as const,
            tc.tile_pool(name="rin", bufs=2) as rin,
            tc.tile_pool(name="exps", bufs=12) as exps,
            tc.tile_pool(name="accp", bufs=2) as accp,
            tc.tile_pool(name="outs", bufs=6) as outs,
            tc.tile_pool(name="psc", bufs=3, space="PSUM") as psc,
            tc.tile_pool(name="pacc", bufs=1, space="PSUM") as pacc,
        ):
            ones_f = const.tile([128, 128], F32)
            nc.vector.memset(ones_f[:], 1.0)
            ones_b = const.tile([128, 128], BF16)
            nc.vector.tensor_copy(ones_b[:], ones_f[:])

            for i in range(BH_PER_CORE):
                q_r = rin.tile([D, S], BF16, tag="q_r")
                k_r = rin.tile([D, S], BF16, tag="k_r")
                v_r = rin.tile([128, T_TILES, D], BF16, tag="v_r")
                # order: what the first tiles need comes first
                nc.sync.dma_start(out=k_r[:, :128], in_=kt[i, :, :128])
                nc.sync.dma_start(out=q_r[:, :512], in_=qt[i, :, :512])
                nc.sync.dma_start(out=q_r[:, 512:S_HALF], in_=qt[i, :, 512:S_HALF])
                nc.sync.dma_start(out=k_r[:, 128:S_HALF], in_=kt[i, :, 128:S_HALF])
                nc.sync.dma_start(
                    out=v_r[:], in_=v[i].rearrange("p (t d) -> p t d", t=T_TILES)
                )
                nc.sync.dma_start(out=q_r[:, S_HALF:], in_=qt[i, :, S_HALF:])
                nc.sync.dma_start(out=k_r[:, S_HALF:], in_=kt[i, :, S_HALF:])

                for h in range(2):
                    s0 = h * S_HALF
                    oacc = pacc.tile([128, S_HALF], F32, tag="oacc")
                    eacc = accp.tile([128, S_HALF], BF16, tag="eacc")

                    ets = [None] * T_TILES

                    def pv(t):
                        for c in range(0, S_HALF, 512):
                            nc.tensor.matmul(
                                oacc[:, c : c + 512],
                                v_r[:, t, :],
                                ets[t][:, c : c + 512],
                                start=(t == 0),
                                stop=(t == T_TILES - 1),
                            )

                    def consume(t):
                        pv(t)
                        if t == 1:
                            nc.vector.tensor_add(eacc[:], ets[0][:], ets[1][:])
                        elif 1 < t < T_TILES - 1:
                            nc.vector.tensor_add(eacc[:], eacc[:], ets[t][:])

                    for t in range(T_TILES):
                        sc = psc.tile([128, S_HALF], F32, tag="sc")
                        for c in range(0, S_HALF, 512):
                            nc.tensor.matmul(
                                sc[:, c : c + 512],
                                k_r[:, t * 128 : (t + 1) * 128],
                                q_r[:, s0 + c : s0 + c + 512],
                                start=True,
                                stop=True,
                            )
                        ets[t] = exps.tile(
                            [128, S_HALF], BF16, tag="et", name=f"et_{t}"
                        )
                        nc.scalar.activation(
                            ets[t][:],
                            sc[:],
                            mybir.ActivationFunctionType.Exp,
                            scale=SCALE,
                        )
                        if t >= 1:
                            consume(t - 1)

                    # tail: PV for the last tile, then the denominator /
                    # normalize chain in 512-col chunks so the sums slot
                    # frees up chunk-by-chunk (~1us after the last exp).
                    tl = T_TILES - 1
                    pv(tl)
                    sacc = psc.tile([128, S_HALF], F32, tag="sc", name="sacc")
                    rec = outs.tile([128, S_HALF], F32, tag="rec")
                    osb = outs.tile([128, S_HALF], F32, tag="osb")
                    for c in range(0, S_HALF, 512):
                        nc.vector.tensor_add(
                            eacc[:, c : c + 512],
                            eacc[:, c : c + 512],
                            ets[tl][:, c : c + 512],
                        )
                        nc.tensor.matmul(
                            sacc[:, c : c + 512],
                            ones_b[:],
                            eacc[:, c : c + 512],
                            start=True,
                            stop=True,
                        )
                        nc.vector.reciprocal_approx_fast(
                            out=rec[:, c : c + 512], in_=sacc[:, c : c + 512]
                        )
                        nc.vector.tensor_mul(
                            osb[:, c : c + 512],
                            oacc[:, c : c + 512],
                            rec[:, c : c + 512],
                        )
                        nc.sync.dma_start(
                            out=ot[i, :, s0 + c : s0 + c + 512],
                            in_=osb[:, c : c + 512],
                        )

    nc.finalize()
    return nc


def _get_program():
    if "nc" not in _cache:
        _cache["nc"] = _build_program()
    return _cache["nc"]


def kernel(q: np.ndarray, k: np.ndarray, v: np.ndarray) -> np.ndarray:
    import ml_dtypes
    from concourse.bass_utils import run_bass_kernel_spmd

    nc = _get_program()

    bf16 = ml_dtypes.bfloat16
    q4 = np.asarray(q, dtype=np.float32).reshape(BH, S, D)
    k4 = np.asarray(k, dtype=np.float32).reshape(BH, S, D)
    v4 = np.asarray(v, dtype=np.float32).reshape(BH, S, D)

    in_maps = []
    for core in range(N_CORES):
        sl = slice(core * BH_PER_CORE, (core + 1) * BH_PER_CORE)
        in_maps.append(
            {
                "qt": np.ascontiguousarray(
                    q4[sl].transpose(0, 2, 1).astype(bf16)
                ),
                "kt": np.ascontiguousarray(
                    k4[sl].transpose(0, 2, 1).astype(bf16)
                ),
                # [i, t*128+p, d] -> [i, p, t*128+d]
                "v": np.ascontiguousarray(
                    v4[sl]
                    .reshape(BH_PER_CORE, T_TILES, 128, D)
                    .transpose(0, 2, 1, 3)
                    .reshape(BH_PER_CORE, 128, T_TILES * D)
                    .astype(bf16)
                ),
            }
        )

    res = run_bass_kernel_spmd(nc, in_maps, core_ids=list(range(N_CORES)))

    out = np.empty((BH, S, D), dtype=np.float32)
    for core in range(N_CORES):
        ot = res.results[core]["ot"]  # [BH_PER_CORE, D, S]
        out[core * BH_PER_CORE : (core + 1) * BH_PER_CORE] = ot.transpose(0, 2, 1)
    return out.reshape(B, H, S, D)


# revision 14
# speedup vs baseline: 1.8420x; 1.0061x over previous
"""Trainium2 Bass kernel for multi-head attention.

Problem: B=4, H=16, S=2048, D=128, fp32.
  scores = (q @ k^T) / sqrt(128); probs = softmax(scores, -1); out = probs @ v

Sharding: 64 (b,h) pairs -> 8 cores x 8 pairs. Fully independent per pair.

v5 layout (all-bf16 datapath; T-layout per (b,h) pair; s in halves of 1024):
  qT, kT: [D=128, S=2048] bf16 in SBUF. For each t-tile (128 keys):
    scoresT[t, s] = kT[:, t-tile].T @ qT   (PE, 2 matmuls N=512, bf16)
    expT = exp(scoresT / sqrt(D)) -> bf16  (ACT, fused scale, psum->sbuf)
    outT[d, s] += v_tile.T @ expT          (PE, 2 matmuls N=512, accum)
    eacc += expT                           (DVE bf16 adds, 2x mode)
  Tail per half, pipelined in 512-column chunks so the psc slot that
  doubles as the sums buffer is released ~1us after the last exp (the
  next half's 3rd QK reuses it; an unchunked chain stalled ACT ~1.9us
  per half in v1/v2):
    chunk c: eacc_c += expT15_c (DVE) -> sums_c = ones.T @ eacc_c (PE)
             -> rec_c = ~1/sums_c (DVE approx) -> out_c = outT_c * rec_c.

PSUM: 3 score slots x 2 banks (one doubles as sums at the tail) + outT
accumulator 2 banks = 8 banks. t-loop software-pipelined by one tile;
exps pool 12 deep so the tail chain never back-pressures ACT.

bf16 end-to-end error vs the fp32 reference is ~3e-3 relative (threshold
2e-2).
"""

import sys

sys.path.insert(0, "/opt/trn_rl_repo")

import numpy as np

B, H, S, D = 4, 16, 2048, 128
N_CORES = 8
BH = B * H                      # 64 pairs
BH_PER_CORE = BH // N_CORES     # 8
T_TILES = S // 128              # 16
S_HALF = S // 2                 # 1024
SCALE = float(D) ** -0.5

_cache = {}


def _build_program():
    import concourse.tile as tile
    from concourse import bacc, mybir

    F32 = mybir.dt.float32
    BF16 = mybir.dt.bfloat16

    nc = bacc.Bacc("TRN2", target_bir_lowering=False, debug=False)

    qt = nc.dram_tensor("qt", [BH_PER_CORE, D, S], BF16, kind="ExternalInput")
    kt = nc.dram_tensor("kt", [BH_PER_CORE, D, S], BF16, kind="ExternalInput")
    # v pre-shuffled on host to [p, t, d] so the load is fully contiguous
    v = nc.dram_tensor("v", [BH_PER_CORE, 128, T_TILES * D], BF16, kind="ExternalInput")
    ot = nc.dram_tensor("ot", [BH_PER_CORE, D, S], BF16, kind="ExternalOutput")

    with tile.TileContext(nc) as tc:
        with (
            tc.tile_pool(name="const", bufs=1) as const,
            tc.tile_pool(name="rin", bufs=2) as rin,
            tc.tile_pool(name="exps", bufs=12) as exps,
            tc.tile_pool(name="accp", bufs=2) as accp,
            tc.tile_pool(name="outs", bufs=6) as outs,
            tc.tile_pool(name="psc", bufs=3, space="PSUM") as psc,
            tc.tile_pool(name="pacc", bufs=1, space="PSUM") as pacc,
        ):
            ones_f = const.tile([128, 128], F32)
            nc.vector.memset(ones_f[:], 1.0)
            ones_b = const.tile([128, 128], BF16)
            nc.vector.tensor_copy(ones_b[:], ones_f[:])

            for i in range(BH_PER_CORE):
                q_r = rin.tile([D, S], BF16, tag="q_r")
                k_r = rin.tile([D, S], BF16, tag="k_r")
                v_r = rin.tile([128, T_TILES, D], BF16, tag="v_r")
                # order: what the first tiles need comes first
                nc.sync.dma_start(out=k_r[:, :128], in_=kt[i, :, :128])
                nc.sync.dma_start(out=q_r[:, :512], in_=qt[i, :, :512])
                nc.sync.dma_start(out=k_r[:, 128:512], in_=kt[i, :, 128:512])
                nc.sync.dma_start(out=q_r[:, 512:S_HALF], in_=qt[i, :, 512:S_HALF])
                nc.sync.dma_start(out=k_r[:, 512:S_HALF], in_=kt[i, :, 512:S_HALF])
                nc.sync.dma_start(
                    out=v_r[:], in_=v[i].rearrange("p (t d) -> p t d", t=T_TILES)
                )
                nc.sync.dma_start(out=q_r[:, S_HALF:], in_=qt[i, :, S_HALF:])
                nc.sync.dma_start(out=k_r[:, S_HALF:], in_=kt[i, :, S_HALF:])

                for h in range(2):
                    s0 = h * S_HALF
                    oacc = pacc.tile([128, S_HALF], F32, tag="oacc")
                    eacc = accp.tile([128, S_HALF], BF16, tag="eacc")

                    ets = [None] * T_TILES

                    def pv(t):
                        for c in range(0, S_HALF, 512):
                            nc.tensor.matmul(
                                oacc[:, c : c + 512],
                                v_r[:, t, :],
                                ets[t][:, c : c + 512],
                                start=(t == 0),
                                stop=(t == T_TILES - 1),
                            )

                    def consume(t):
                        pv(t)
                        if t == 1:
                            nc.vector.tensor_add(eacc[:], ets[0][:], ets[1][:])
                        elif 1 < t < T_TILES - 1:
                            nc.vector.tensor_add(eacc[:], eacc[:], ets[t][:])

                    for t in range(T_TILES):
                        sc = psc.tile([128, S_HALF], F32, tag="sc")
                        for c in range(0, S_HALF, 512):
                            nc.tensor.matmul(
                                sc[:, c : c + 512],
                                k_r[:, t * 128 : (t + 1) * 128],
                                q_r[:, s0 + c : s0 + c + 512],
                                start=True,
                                stop=True,
                            )
                        ets[t] = exps.tile(
                            [128, S_HALF], BF16, tag="et", name=f"et_{t}"
                        )
                        nc.scalar.activation(
                            ets[t][:],
                            sc[:],
                            mybir.ActivationFunctionType.Exp,
                            scale=SCALE,
                        )
                        if t >= 1:
                            consume(t - 1)

                    # tail: PV for the last tile, then the denominator /
                    # normalize chain in 512-col chunks so the sums slot
                    # frees up chunk-by-chunk (~1us after the last exp).
                    tl = T_TILES - 1
                    pv(tl)
                    sacc = psc.tile([128, S_HALF], F32, tag="sc", name="sacc")
                    rec = outs.tile([128, S_HALF], F32, tag="rec")
                    osb = outs.tile([128, S_HALF], BF16, tag="osb")
                    # sums pass 1: tiles 0..14 (ready at exp14) -- runs on
                    # PE during exp15, keeping the last DVE add out of the
                    # slot-release chain.
                    for c in range(0, S_HALF, 512):
                        nc.tensor.matmul(
                            sacc[:, c : c + 512],
                            ones_b[:],
                            eacc[:, c : c + 512],
                            start=True,
                            stop=False,
                        )
                    # sums pass 2 + normalize, chunked: chain after exp15
                    # is just matmul -> recip per 512 cols.
                    for c in range(0, S_HALF, 512):
                        nc.tensor.matmul(
                            sacc[:, c : c + 512],
                            ones_b[:],
                            ets[tl][:, c : c + 512],
                            start=False,
                            stop=True,
                        )
                        nc.vector.reciprocal_approx_fast(
                            out=rec[:, c : c + 512], in_=sacc[:, c : c + 512]
                        )
                        nc.vector.tensor_mul(
                            osb[:, c : c + 512],
                            oacc[:, c : c + 512],
                            rec[:, c : c + 512],
                        )
                        nc.sync.dma_start(
                            out=ot[i, :, s0 + c : s0 + c + 512],
                            in_=osb[:, c : c + 512],
                        )

    nc.finalize()
    return nc


def _get_program():
    if "nc" not in _cache:
        _cache["nc"] = _build_program()
    return _cache["nc"]


def kernel(q: np.ndarray, k: np.ndarray, v: np.ndarray) -> np.ndarray:
    import ml_dtypes
    from concourse.bass_utils import run_bass_kernel_spmd

    nc = _get_program()

    bf16 = ml_dtypes.bfloat16
    q4 = np.asarray(q, dtype=np.float32).reshape(BH, S, D)
    k4 = np.asarray(k, dtype=np.float32).reshape(BH, S, D)
    v4 = np.asarray(v, dtype=np.float32).reshape(BH, S, D)

    in_maps = []
    for core in range(N_CORES):
        sl = slice(core * BH_PER_CORE, (core + 1) * BH_PER_CORE)
        in_maps.append(
            {
                "qt": np.ascontiguousarray(
                    q4[sl].transpose(0, 2, 1).astype(bf16)
                ),
                "kt": np.ascontiguousarray(
                    k4[sl].transpose(0, 2, 1).astype(bf16)
                ),
                # [i, t*128+p, d] -> [i, p, t*128+d]
                "v": np.ascontiguousarray(
                    v4[sl]
                    .reshape(BH_PER_CORE, T_TILES, 128, D)
                    .transpose(0, 2, 1, 3)
                    .reshape(BH_PER_CORE, 128, T_TILES * D)
                    .astype(bf16)
                ),
            }
        )

    res = run_bass_kernel_spmd(nc, in_maps, core_ids=list(range(N_CORES)))

    out = np.empty((BH, S, D), dtype=np.float32)
    for core in range(N_CORES):
        ot = res.results[core]["ot"]  # [BH_PER_CORE, D, S] bf16
        out[core * BH_PER_CORE : (core + 1) * BH_PER_CORE] = (
            ot.transpose(0, 2, 1).astype(np.float32)
        )
    return out.reshape(B, H, S, D)


# revision 15
# speedup vs baseline: 1.9455x; 1.0561x over previous
"""Trainium2 Bass kernel for multi-head attention.

Problem: B=4, H=16, S=2048, D=128, fp32.
  scores = (q @ k^T) / sqrt(128); probs = softmax(scores, -1); out = probs @ v

Sharding: 64 (b,h) pairs -> 8 cores x 8 pairs. Fully independent per pair.

v7 layout (all-bf16 datapath; T-layout per (b,h) pair; s in halves of 1024):
  qT, kT: [D=128, S=2048] bf16 in SBUF. For each t-tile (128 keys):
    scoresT[t, s] = kT[:, t-tile].T @ qT   (PE, 2 matmuls N=512, bf16)
    expT = exp(scoresT / sqrt(D)) -> bf16  (ACT, fused scale, psum->sbuf)
    outT[d, s] += v_tile.T @ expT          (PE, 2 matmuls N=512, accum)
    eacc += expT                           (DVE bf16 adds, 2x mode)
  Denominator: eacc (tiles 0..14) collapsed over the 128 t-partitions by a
  PE ones-matmul, tile 15 folded by a second accumulating ones-matmul;
  reciprocal (DVE approx), out = outT * rec (DVE), DMA out bf16 (host
  upcasts to fp32).

  The whole tail of half h (PV of tiles 14..15, sums, normalize, store) is
  EMITTED inside half h+1's t-loop after its second exp: the PE executes
  its queue in order, so this puts the next half's first QKs ahead of the
  exp15-gated tail matmuls -- ACT never idles at half boundaries (this
  was a ~1.5us stall per half in earlier versions). The PV consume lag is
  2 tiles to match.

PSUM: 3 score slots x 2 banks (one doubles as the sums buffer during the
deferred tail) + outT accumulator 2 banks = 8 banks.

bf16 end-to-end error vs the fp32 reference is ~3.5e-3 relative
(threshold 2e-2).
"""

import sys

sys.path.insert(0, "/opt/trn_rl_repo")

import numpy as np

B, H, S, D = 4, 16, 2048, 128
N_CORES = 8
BH = B * H                      # 64 pairs
BH_PER_CORE = BH // N_CORES     # 8
T_TILES = S // 128              # 16
S_HALF = S // 2                 # 1024
SCALE = float(D) ** -0.5

_cache = {}


def _build_program():
    import concourse.tile as tile
    from concourse import bacc, mybir

    F32 = mybir.dt.float32
    BF16 = mybir.dt.bfloat16

    nc = bacc.Bacc("TRN2", target_bir_lowering=False, debug=False)

    qt = nc.dram_tensor("qt", [BH_PER_CORE, D, S], BF16, kind="ExternalInput")
    kt = nc.dram_tensor("kt", [BH_PER_CORE, D, S], BF16, kind="ExternalInput")
    # v pre-shuffled on host to [p, t, d] so the load is fully contiguous
    v = nc.dram_tensor("v", [BH_PER_CORE, 128, T_TILES * D], BF16, kind="ExternalInput")
    ot = nc.dram_tensor("ot", [BH_PER_CORE, D, S], BF16, kind="ExternalOutput")

    with tile.TileContext(nc) as tc:
        with (
            tc.tile_pool(name="const", bufs=1) as const,
            tc.tile_pool(name="rin", bufs=2) as rin,
            tc.tile_pool(name="exps", bufs=12) as exps,
            tc.tile_pool(name="accp", bufs=2) as accp,
            tc.tile_pool(name="outs", bufs=6) as outs,
            tc.tile_pool(name="psc", bufs=3, space="PSUM") as psc,
            tc.tile_pool(name="pacc", bufs=1, space="PSUM") as pacc,
        ):
            ones_f = const.tile([128, 128], F32)
            nc.vector.memset(ones_f[:], 1.0)
            ones_b = const.tile([128, 128], BF16)
            nc.vector.tensor_copy(ones_b[:], ones_f[:])

            pending_tail = [None]

            def flush_tail():
                if pending_tail[0] is not None:
                    t_fn = pending_tail[0]
                    pending_tail[0] = None
                    t_fn()

            for i in range(BH_PER_CORE):
                q_r = rin.tile([D, S], BF16, tag="q_r")
                k_r = rin.tile([D, S], BF16, tag="k_r")
                v_r = rin.tile([128, T_TILES, D], BF16, tag="v_r")
                # order: what the first tiles need comes first; v split so
                # the first PV tiles don't wait on one huge transfer
                nc.sync.dma_start(out=k_r[:, :128], in_=kt[i, :, :128])
                nc.sync.dma_start(out=q_r[:, :512], in_=qt[i, :, :512])
                nc.sync.dma_start(out=k_r[:, 128:512], in_=kt[i, :, 128:512])
                vv = v[i].rearrange("p (t d) -> p t d", t=T_TILES)
                nc.sync.dma_start(out=v_r[:, 0:4], in_=vv[:, 0:4])
                nc.sync.dma_start(out=q_r[:, 512:S_HALF], in_=qt[i, :, 512:S_HALF])
                nc.sync.dma_start(out=k_r[:, 512:S_HALF], in_=kt[i, :, 512:S_HALF])
                nc.sync.dma_start(out=v_r[:, 4:8], in_=vv[:, 4:8])
                nc.sync.dma_start(out=v_r[:, 8:12], in_=vv[:, 8:12])
                nc.sync.dma_start(out=v_r[:, 12:16], in_=vv[:, 12:16])
                nc.sync.dma_start(out=q_r[:, S_HALF:], in_=qt[i, :, S_HALF:])
                nc.sync.dma_start(out=k_r[:, S_HALF:], in_=kt[i, :, S_HALF:])

                for h in range(2):
                    s0 = h * S_HALF
                    oacc = pacc.tile([128, S_HALF], F32, tag="oacc")
                    eacc = accp.tile([128, S_HALF], BF16, tag="eacc")

                    ets = [None] * T_TILES

                    def pv(t, oacc=oacc, v_r=v_r, ets=ets):
                        for c in range(0, S_HALF, 512):
                            nc.tensor.matmul(
                                oacc[:, c : c + 512],
                                v_r[:, t, :],
                                ets[t][:, c : c + 512],
                                start=(t == 0),
                                stop=(t == T_TILES - 1),
                            )

                    def consume(t, eacc=eacc, ets=ets, pv=pv):
                        pv(t)
                        if t == 1:
                            nc.vector.tensor_add(eacc[:], ets[0][:], ets[1][:])
                        elif 1 < t < T_TILES - 1:
                            nc.vector.tensor_add(eacc[:], eacc[:], ets[t][:])

                    for t in range(T_TILES):
                        sc = psc.tile([128, S_HALF], F32, tag="sc")
                        for c in range(0, S_HALF, 512):
                            nc.tensor.matmul(
                                sc[:, c : c + 512],
                                k_r[:, t * 128 : (t + 1) * 128],
                                q_r[:, s0 + c : s0 + c + 512],
                                start=True,
                                stop=True,
                            )
                        ets[t] = exps.tile(
                            [128, S_HALF], BF16, tag="et", name=f"et_{t}"
                        )
                        nc.scalar.activation(
                            ets[t][:],
                            sc[:],
                            mybir.ActivationFunctionType.Exp,
                            scale=SCALE,
                        )
                        if t == 1:
                            # previous half's tail lands here: after this
                            # half's first two QKs in PE program order
                            flush_tail()
                        if t >= 2:
                            consume(t - 2)

                    def make_tail(i=i, s0=s0, oacc=oacc, eacc=eacc,
                                  ets=ets, pv=pv, consume=consume):
                        def tail():
                            consume(T_TILES - 2)   # PV14 + add14
                            pv(T_TILES - 1)        # PV15
                            sacc = psc.tile(
                                [128, S_HALF], F32, tag="sc", name="sacc"
                            )
                            rec = outs.tile([128, S_HALF], F32, tag="rec")
                            osb = outs.tile([128, S_HALF], BF16, tag="osb")
                            for c in range(0, S_HALF, 512):
                                nc.tensor.matmul(
                                    sacc[:, c : c + 512],
                                    ones_b[:],
                                    eacc[:, c : c + 512],
                                    start=True,
                                    stop=False,
                                )
                            for c in range(0, S_HALF, 512):
                                nc.tensor.matmul(
                                    sacc[:, c : c + 512],
                                    ones_b[:],
                                    ets[T_TILES - 1][:, c : c + 512],
                                    start=False,
                                    stop=True,
                                )
                                nc.vector.reciprocal_approx_fast(
                                    out=rec[:, c : c + 512],
                                    in_=sacc[:, c : c + 512],
                                )
                                nc.vector.tensor_mul(
                                    osb[:, c : c + 512],
                                    oacc[:, c : c + 512],
                                    rec[:, c : c + 512],
                                )
                                nc.sync.dma_start(
                                    out=ot[i, :, s0 + c : s0 + c + 512],
                                    in_=osb[:, c : c + 512],
                                )
                        return tail

                    pending_tail[0] = make_tail()

            flush_tail()

    nc.finalize()
    return nc


def _get_program():
    if "nc" not in _cache:
        _cache["nc"] = _build_program()
    return _cache["nc"]


def kernel(q: np.ndarray, k: np.ndarray, v: np.ndarray) -> np.ndarray:
    import ml_dtypes
    from concourse.bass_utils import run_bass_kernel_spmd

    nc = _get_program()

    bf16 = ml_dtypes.bfloat16
    q4 = np.asarray(q, dtype=np.float32).reshape(BH, S, D)
    k4 = np.asarray(k, dtype=np.float32).reshape(BH, S, D)
    v4 = np.asarray(v, dtype=np.float32).reshape(BH, S, D)

    in_maps = []
    for core in range(N_CORES):
        sl = slice(core * BH_PER_CORE, (core + 1) * BH_PER_CORE)
        in_maps.append(
            {
                "qt": np.ascontiguousarray(
                    q4[sl].transpose(0, 2, 1).astype(bf16)
                ),
                "kt": np.ascontiguousarray(
                    k4[sl].transpose(0, 2, 1).astype(bf16)
                ),
                # [i, t*128+p, d] -> [i, p, t*128+d]
                "v": np.ascontiguousarray(
                    v4[sl]
                    .reshape(BH_PER_CORE, T_TILES, 128, D)
                    .transpose(0, 2, 1, 3)
                    .reshape(BH_PER_CORE, 128, T_TILES * D)
                    .astype(bf16)
                ),
            }
        )

    res = run_bass_kernel_spmd(nc, in_maps, core_ids=list(range(N_CORES)))

    out = np.empty((BH, S, D), dtype=np.float32)
    for core in range(N_CORES):
        ot = res.results[core]["ot"]  # [BH_PER_CORE, D, S] bf16
        out[core * BH_PER_CORE : (core + 1) * BH_PER_CORE] = (
            ot.transpose(0, 2, 1).astype(np.float32)
        )
    return out.reshape(B, H, S, D)
